# revision 39
# baseline (speedup 1.0000x reference)
"""Trainium2 Bass kernel for cross-attention with per-head structured mask.

Reference computation (B=4, N=1024, DIM=1024, H=16, D=64):
    q = x1 @ Wq;  k, v = split(x2 @ Wkv)
    dots = q k^T * D^-0.5 + spd
    attn = softmax(dots) * (head_keep * H / n_kept)   # whole heads dropped
    out  = (attn @ v) @ Wo + bo

Sharding: dropped heads contribute exactly zero, so only kept heads are
computed. Work unit = (batch b, kept-head group g): 8 cores = 4 batches x 2
head groups. Each core computes a partial out[b] (its heads' contribution
through Wo); host sums the two partials per batch and adds the bias.

Device layout (per core, H_c heads):
    QT[hd, n], KT[hd, m] via PE (contraction over DIM, inputs pre-transposed
    on host).  V held as [m, h, 128] blocks: per head, 64 cols of V plus 64
    cols of ones (parity-swapped), so ctx_psum = V_aug^T @ exp(scores^T)
    carries both the context rows AND the softmax denominator rows in one
    accumulation, landing at the partition base the final ctxT layout needs.

HW quirks baked in (found empirically):
  - custom-DVE ops (reciprocal_approx_fast) and K=1 matmul operands only
    behave at partition base 0 -> shuttle rows down via tiny DMAs.
  - fp32/fp32r matmuls run at ~2-4 cyc/row; bf16 runs at 1 cyc/row, so
    matmul operands default to bf16 (PSUM accumulation stays fp32).
"""

import os

import numpy as np

B, N, DIM = 4, 1024, 1024
HEADS, DIM_HEAD = 16, 64
INNER = HEADS * DIM_HEAD
SCALE = DIM_HEAD ** -0.5
NCORES = 8
KT = DIM // 128      # 8 contraction tiles
NB = N // 512        # 2 column blocks
MT = N // 128        # 8 key tiles

_cache: dict = {}


def _build(H_c: int, keep_scale: float, mode: str = "bf16", half_last: bool = False):
    """Build + compile the per-core Bass program for H_c heads (H_c even)."""
    import concourse.mybir as mybir
    import concourse.tile as tile
    from concourse import bacc

    dt = mybir.dt
    f32 = dt.float32
    HB = H_c // 2
    HD = H_c * DIM_HEAD
    assert H_c % 2 == 0 and HD <= 512

    mmdt = {"bf16": dt.bfloat16, "f32r": dt.float32r, "f32": f32}[mode]

    nc = bacc.Bacc("TRN2", target_bir_lowering=False)

    xq = nc.dram_tensor("xq", [128, KT, N], mmdt, kind="ExternalInput")   # x1[b].T, sbuf image
    xk = nc.dram_tensor("xk", [128, KT, N], mmdt, kind="ExternalInput")   # x2[b].T, sbuf image
    wq = nc.dram_tensor("wq", [128, KT, HD], mmdt, kind="ExternalInput")  # sbuf layout
    wk = nc.dram_tensor("wk", [128, KT, HD], mmdt, kind="ExternalInput")
    wv = nc.dram_tensor("wv", [128, KT, HD], mmdt, kind="ExternalInput")
    wo = nc.dram_tensor("wo", [128, HD // 128, DIM], mmdt, kind="ExternalInput")
    # spd[b,h].T in [n-block, partition, m-tile, n] sbuf-image layout
    spd = nc.dram_tensor("spd", [H_c, NB, 128, MT, 512], mmdt, kind="ExternalInput")
    eye = nc.dram_tensor("eye", [128, 128], mmdt, kind="ExternalInput")
    out = nc.dram_tensor("out", [N, DIM], f32, kind="ExternalOutput")

    Exp = mybir.ActivationFunctionType.Exp
    mult = mybir.AluOpType.mult

    with tile.TileContext(nc) as tc:
        with (
            tc.tile_pool(name="w", bufs=1) as wpool,
            tc.tile_pool(name="big", bufs=1) as big,
            tc.tile_pool(name="spdp", bufs=3) as spdp,
            tc.tile_pool(name="work", bufs=4) as work,
            tc.tile_pool(name="psA", bufs=2, space="PSUM") as psA,
            tc.tile_pool(name="psS", bufs=4, space="PSUM") as psS,
            tc.tile_pool(name="psC", bufs=2, space="PSUM") as psC,
        ):
            wq_sb = wpool.tile([128, KT, HD], mmdt, tag="wq")
            wk_sb = wpool.tile([128, KT, HD], mmdt, tag="wk")
            wv_sb = wpool.tile([128, KT, HD], mmdt, tag="wv")
            wo_sb = wpool.tile([128, HD // 128, DIM], mmdt, tag="wo")
            eye_sb = wpool.tile([128, 128], mmdt, tag="eye")
            # x2T image goes first, k-chunked so the first KT matmul can
            # start after ~0.4MB instead of 2.5MB
            xk_sb = big.tile([128, KT, N], mmdt, tag="xkim")
            nc.sync.dma_start(wk_sb[:, 0, :], wk[:, 0, :])
            nc.sync.dma_start(xk_sb[:, 0, :], xk[:, 0, :])
            nc.sync.dma_start(wk_sb[:, 1:, :], wk[:, 1:, :])
            for k in range(1, KT):
                nc.sync.dma_start(xk_sb[:, k, :], xk[:, k, :])
            nc.sync.dma_start(wv_sb[:], wv[:])
            nc.sync.dma_start(eye_sb[:], eye[:])
            nc.sync.dma_start(wq_sb[:], wq[:])
            xq_sb = big.tile([128, KT, N], mmdt, tag="xqim")
            nc.sync.dma_start(xq_sb[:], xq[:])
            nc.sync.dma_start(wo_sb[:], wo[:])

            qt_sb = big.tile([128, HB, N], mmdt, tag="qt")
            kt_sb = big.tile([128, HB, N], mmdt, tag="kt")
            v_sb = big.tile([128, MT, H_c * 128], mmdt, tag="v")
            ct_sb = big.tile([128, HB, N], mmdt, tag="ct")

            if half_last:
                # the shared head's slot is only computed at local n0=0; its
                # n0=1 region of ctxT must read as zero in the out projection
                nc.gpsimd.memset(ct_sb[64:128, HB - 1, 512:N], 0.0)

            # ones columns of the augmented V blocks (parity-swapped per head)
            for h in range(H_c):
                c0 = h * 128 + (64 if h % 2 == 0 else 0)
                for m in range(MT):
                    nc.gpsimd.memset(v_sb[:, m, c0:c0 + 64], 1.0)

            # ---- Phase A: projections. One 1MB slab DMA per 512-column
            # block; KT and V share the x2T slabs. ----
            def v_copyout(ps_ap, m):
                pv = ps_ap.rearrange("p (hb two d) -> p hb two d", two=2, d=64)
                vv = v_sb[:, m, :].rearrange("p (hb x) -> p hb x", hb=HB)
                # even heads -> value cols 0:64 of their block; odd -> 192:256
                nc.vector.tensor_copy(vv[:, :, 0:64], pv[:, :, 0, :])
                nc.vector.tensor_copy(vv[:, :, 192:256], pv[:, :, 1, :])

            for m0 in range(NB):
                m_sl = slice(m0 * 512, (m0 + 1) * 512)
                slab = xk_sb[:, :, m_sl]
                kps = [psA.tile([128, 512], f32, tag="acc",
                                name=f"kps{m0}_{i}") for i in range(HB)]
                vps = [psS.tile([128, HD], f32, tag="sc",
                                name=f"vps{m0}_{i}") for i in range(2)]
                for k in range(KT):
                    for hb in range(HB):
                        nc.tensor.matmul(
                            kps[hb][:],
                            wk_sb[:, k, hb * 128:(hb + 1) * 128],
                            slab[:, k, :],
                            start=(k == 0), stop=(k == KT - 1),
                        )
                    for mi in range(2):
                        nc.tensor.matmul(
                            vps[mi][:],
                            slab[:, k, mi * 128:(mi + 1) * 128],
                            wv_sb[:, k, :],
                            start=(k == 0), stop=(k == KT - 1),
                        )
                for hb in range(HB):
                    nc.vector.tensor_copy(kt_sb[:, hb, m_sl], kps[hb][:])
                for mi in range(2):
                    v_copyout(vps[mi][:], m0 * 4 + mi)
                # second half of the V m-tiles from the same slab
                vpsb = [psS.tile([128, HD], f32, tag="sc",
                                 name=f"vpsb{m0}_{i}") for i in range(2)]
                for k in range(KT):
                    for mi in range(2):
                        nc.tensor.matmul(
                            vpsb[mi][:],
                            slab[:, k, (2 + mi) * 128:(3 + mi) * 128],
                            wv_sb[:, k, :],
                            start=(k == 0), stop=(k == KT - 1),
                        )
                for mi in range(2):
                    v_copyout(vpsb[mi][:], m0 * 4 + 2 + mi)

            for n0 in range(NB):
                n_sl = slice(n0 * 512, (n0 + 1) * 512)
                slab = xq_sb[:, :, n_sl]
                qps = [psA.tile([128, 512], f32, tag="acc",
                                name=f"psq{n0}_{i}") for i in range(HB)]
                for k in range(KT):
                    for hb in range(HB):
                        nc.tensor.matmul(
                            qps[hb][:],
                            wq_sb[:, k, hb * 128:(hb + 1) * 128],
                            slab[:, k, :],
                            start=(k == 0), stop=(k == KT - 1),
                        )
                for hb in range(HB):
                    nc.vector.tensor_copy(qt_sb[:, hb, n_sl], qps[hb][:])

            # ---- Phase B (attention) + C (out proj), interleaved per n-block ----
            for n0 in range(NB):
                n_sl = slice(n0 * 512, (n0 + 1) * 512)
                nheads = H_c - 1 if (half_last and n0 > 0) else H_c
                for h in range(nheads):
                    hb, hp = divmod(h, 2)
                    vb = hp * 64          # partition base of ctx values
                    sb_ = 64 - vb         # partition base of sumexp rows
                    ctx = psC.tile([128, 512], f32, tag="ctx")
                    spds = spdp.tile([128, MT, 512], mmdt, tag="spd")
                    nc.sync.dma_start(spds[:], spd[h, n0])
                    for m in range(MT):
                        sc = psS.tile([128, 512], f32, tag="sc")
                        nc.tensor.matmul(
                            sc[:],
                            kt_sb[vb:vb + 64, hb, m * 128:(m + 1) * 128],
                            qt_sb[vb:vb + 64, hb, n_sl],
                            start=True, stop=False,
                        )
                        # accumulate spd into the scores PSUM via identity MM
                        nc.tensor.matmul(
                            sc[:], eye_sb[:], spds[:, m, :], start=False, stop=True,
                        )
                        e = work.tile([128, 512], mmdt, tag="e")
                        nc.scalar.activation(e[:], sc[:], Exp)
                        nc.tensor.matmul(
                            ctx[:],
                            v_sb[:, m, h * 128:(h + 1) * 128],
                            e[:],
                            start=(m == 0),
                            stop=(m == MT - 1),
                        )
                    # normalize: ctxT = ctx_vals * keep_scale / sumexp (the
                    # ones block replicated sumexp across 64 rows at base sb_;
                    # copy to base 0 -- custom-DVE recip needs base 0 -- then
                    # stt against the PSUM value rows).
                    rr = work.tile([128, 512], f32, tag="rr")
                    if sb_ == 0:
                        nc.vector.reciprocal_approx_fast(
                            rr[0:64, :], ctx[0:64, :])
                    else:
                        ss = work.tile([128, 512], f32, tag="ss")
                        nc.vector.tensor_copy(ss[0:64, :], ctx[sb_:sb_ + 64, :])
                        nc.vector.reciprocal_approx_fast(rr[0:64, :], ss[0:64, :])
                    nc.vector.scalar_tensor_tensor(
                        out=ct_sb[vb:vb + 64, hb, n_sl],
                        in0=ctx[vb:vb + 64, :],
                        scalar=float(keep_scale),
                        in1=rr[0:64, :],
                        op0=mult,
                        op1=mult,
                    )

                # ---- Phase C for this n-block ----
                for nt in range(n0 * 4, (n0 + 1) * 4):
                    ot = work.tile([128, 2, 512], f32, tag="o")
                    for d0 in range(NB):
                        po = psA.tile([128, 512], f32, tag="acc",
                                      name=f"po{nt}_{d0}")
                        for kk in range(HD // 128):
                            nc.tensor.matmul(
                                po[:],
                                ct_sb[:, kk, nt * 128:(nt + 1) * 128],
                                wo_sb[:, kk, d0 * 512:(d0 + 1) * 512],
                                start=(kk == 0),
                                stop=(kk == HD // 128 - 1),
                            )
                        nc.vector.tensor_copy(ot[:, d0, :], po[:])
                    nc.sync.dma_start(
                        out[nt * 128:(nt + 1) * 128, :], ot[:])

    nc.finalize()
    return nc


def _get_nc(H_c: int, n_kept: int, mode: str, half_last: bool):
    key = (H_c, n_kept, mode, half_last)
    if key not in _cache:
        _cache[key] = _build(H_c, HEADS / n_kept, mode, half_last)
    return _cache[key]


def _prep_inputs(x1, x2, spd, head_keep, Wq, Wkv, Wo, mode="bf16"):
    """Slice/transpose/pad host-side into per-core input maps."""
    import ml_dtypes

    ndt = np.float32 if mode in ("f32", "f32r") else ml_dtypes.bfloat16
    kept = [int(i) for i in np.nonzero(head_keep)[0]]
    n_kept = len(kept)
    half_last = (n_kept % 2 == 1)
    if not half_last:
        H_c = n_kept // 2
        if H_c % 2:
            H_c += 1
        groups = [kept[:H_c], kept[H_c:]]
    else:
        # odd count: both cores of a pair share the last kept head, each
        # computing one n-half of it (local column order differs per core)
        K = (n_kept - 1) // 2
        shared = kept[-1]
        H_c = K + 1
        pad = []
        if H_c % 2:
            H_c += 1
            pad = [None]
        groups = [kept[:K] + pad + [shared], kept[K:2 * K] + pad + [shared]]

    Wk_full, Wv_full = Wkv[:, :INNER], Wkv[:, INNER:]

    in_maps = []
    for b in range(B):
        xqT = np.ascontiguousarray(
            x1[b].T.reshape(KT, 128, N).transpose(1, 0, 2)).astype(ndt)
        xkT = np.ascontiguousarray(
            x2[b].T.reshape(KT, 128, N).transpose(1, 0, 2)).astype(ndt)
        for g in range(2):
            heads = groups[g]
            swap = half_last and g == 1  # local n0=0 <-> global half 1
            xq_g = xqT
            if swap:
                xq_g = np.ascontiguousarray(
                    np.concatenate([xqT[:, :, 512:], xqT[:, :, :512]], axis=2))
            HD = H_c * DIM_HEAD
            wq_c = np.zeros((DIM, HD), np.float32)
            wk_c = np.zeros((DIM, HD), np.float32)
            wv_c = np.zeros((DIM, HD), np.float32)
            wo_c = np.zeros((HD, DIM), np.float32)
            spd_c = np.zeros((H_c, NB, 128, MT, 512), ndt)
            for i, h in enumerate(heads):
                if h is None:
                    continue
                sl = slice(i * DIM_HEAD, (i + 1) * DIM_HEAD)
                hs = slice(h * DIM_HEAD, (h + 1) * DIM_HEAD)
                wq_c[:, sl] = Wq[:, hs] * SCALE
                wk_c[:, sl] = Wk_full[:, hs]
                wv_c[:, sl] = Wv_full[:, hs]
                wo_c[sl, :] = Wo[hs, :]
                # spd[b,h].T -> [n-block, partition, m-tile, n] image,
                # n-blocks in the core's LOCAL column order
                im = (spd[b, h].T.reshape(MT, 128, NB, 512)
                      .transpose(2, 1, 0, 3)).astype(ndt)
                spd_c[i] = im[::-1] if swap else im
            in_maps.append({
                "xq": xq_g,
                "xk": xkT,
                "eye": np.eye(128, dtype=ndt),
                "wq": np.ascontiguousarray(
                    wq_c.reshape(KT, 128, HD).transpose(1, 0, 2)).astype(ndt),
                "wk": np.ascontiguousarray(
                    wk_c.reshape(KT, 128, HD).transpose(1, 0, 2)).astype(ndt),
                "wv": np.ascontiguousarray(
                    wv_c.reshape(KT, 128, HD).transpose(1, 0, 2)).astype(ndt),
                "wo": np.ascontiguousarray(
                    wo_c.reshape(HD // 128, 128, DIM).transpose(1, 0, 2)).astype(ndt),
                "spd": spd_c,
            })
    return in_maps, n_kept, H_c


def _run(nc, in_maps, trace=False, tmpdir=None):
    from concourse.bass_utils import run_bass_kernel_spmd

    return run_bass_kernel_spmd(
        nc, in_maps, core_ids=list(range(NCORES)), trace=trace, tmpdir=tmpdir
    )


def kernel(x1, x2, spd, head_keep, Wq, Wkv, Wo, bo, _trace=False, _tmpdir=None):
    x1 = np.asarray(x1, np.float32)
    x2 = np.asarray(x2, np.float32)
    spd = np.asarray(spd, np.float32)
    head_keep = np.asarray(head_keep)
    n_kept = int(head_keep.astype(np.int64).sum())
    if n_kept == 0:
        # reference: 16/0 = inf, 0*inf = nan everywhere
        return np.full((B, N, DIM), np.nan, np.float32)

    mode = os.environ.get("KERNEL_DTYPE", "bf16")
    in_maps, n_kept, H_c = _prep_inputs(
        x1, x2, spd, head_keep, Wq, Wkv, Wo, mode)
    half_last = (n_kept % 2 == 1)
    nc = _get_nc(H_c, n_kept, mode, half_last)
    res = _run(nc, in_maps, trace=_trace, tmpdir=_tmpdir)

    out = np.empty((B, N, DIM), np.float32)
    bo32 = np.asarray(bo, np.float32)
    for b in range(B):
        o0 = res.results[2 * b]["out"]
        o1 = res.results[2 * b + 1]["out"]
        if half_last:
            o1 = np.concatenate([o1[512:], o1[:512]], axis=0)
        out[b] = o0 + o1 + bo32
    kernel._last_results = res
    return out


# revision 40
# speedup vs baseline: 1.0263x; 1.0263x over previous
"""Trainium2 Bass kernel for cross-attention with per-head structured mask.

Reference computation (B=4, N=1024, DIM=1024, H=16, D=64):
    q = x1 @ Wq;  k, v = split(x2 @ Wkv)
    dots = q k^T * D^-0.5 + spd
    attn = softmax(dots) * (head_keep * H / n_kept)   # whole heads dropped
    out  = (attn @ v) @ Wo + bo

Sharding: dropped heads contribute exactly zero, so only kept heads are
computed. Work unit = (batch b, kept-head group g): 8 cores = 4 batches x 2
head groups. Each core computes a partial out[b] (its heads' contribution
through Wo); host sums the two partials per batch and adds the bias.

Device layout (per core, H_c heads):
    QT[hd, n], KT[hd, m] via PE (contraction over DIM, inputs pre-transposed
    on host).  V held as [m, h, 128] blocks: per head, 64 cols of V plus 64
    cols of ones (parity-swapped), so ctx_psum = V_aug^T @ exp(scores^T)
    carries both the context rows AND the softmax denominator rows in one
    accumulation, landing at the partition base the final ctxT layout needs.

HW quirks baked in (found empirically):
  - custom-DVE ops (reciprocal_approx_fast) and K=1 matmul operands only
    behave at partition base 0 -> shuttle rows down via tiny DMAs.
  - fp32/fp32r matmuls run at ~2-4 cyc/row; bf16 runs at 1 cyc/row, so
    matmul operands default to bf16 (PSUM accumulation stays fp32).
"""

import os

import numpy as np

B, N, DIM = 4, 1024, 1024
HEADS, DIM_HEAD = 16, 64
INNER = HEADS * DIM_HEAD
SCALE = DIM_HEAD ** -0.5
NCORES = 8
KT = DIM // 128      # 8 contraction tiles
NB = N // 512        # 2 column blocks
MT = N // 128        # 8 key tiles

_cache: dict = {}


def _build(H_c: int, keep_scale: float, mode: str = "bf16", half_last: bool = False):
    """Build + compile the per-core Bass program for H_c heads (H_c even)."""
    import concourse.mybir as mybir
    import concourse.tile as tile
    from concourse import bacc

    dt = mybir.dt
    f32 = dt.float32
    HB = H_c // 2
    HD = H_c * DIM_HEAD
    assert H_c % 2 == 0 and HD <= 512

    mmdt = {"bf16": dt.bfloat16, "f32r": dt.float32r, "f32": f32}[mode]

    nc = bacc.Bacc("TRN2", target_bir_lowering=False)

    xq = nc.dram_tensor("xq", [128, KT, N], mmdt, kind="ExternalInput")   # x1[b].T, sbuf image
    xk = nc.dram_tensor("xk", [128, KT, N], mmdt, kind="ExternalInput")   # x2[b].T, sbuf image
    wq = nc.dram_tensor("wq", [128, KT, HD], mmdt, kind="ExternalInput")  # sbuf layout
    wk = nc.dram_tensor("wk", [128, KT, HD], mmdt, kind="ExternalInput")
    wv = nc.dram_tensor("wv", [128, KT, HD], mmdt, kind="ExternalInput")
    wo = nc.dram_tensor("wo", [128, HD // 128, DIM], mmdt, kind="ExternalInput")
    # spd[b,h].T in [n-block, partition, m-tile, n] sbuf-image layout
    spd = nc.dram_tensor("spd", [H_c, NB, 128, MT, 512], mmdt, kind="ExternalInput")
    eye = nc.dram_tensor("eye", [128, 128], mmdt, kind="ExternalInput")
    out = nc.dram_tensor("out", [N, DIM], f32, kind="ExternalOutput")

    Exp = mybir.ActivationFunctionType.Exp
    mult = mybir.AluOpType.mult

    with tile.TileContext(nc) as tc:
        with (
            tc.tile_pool(name="w", bufs=1) as wpool,
            tc.tile_pool(name="big", bufs=1) as big,
            tc.tile_pool(name="spdp", bufs=3) as spdp,
            tc.tile_pool(name="work", bufs=4) as work,
            tc.tile_pool(name="psA", bufs=3, space="PSUM") as psA,
            tc.tile_pool(name="psS", bufs=3, space="PSUM") as psS,
            tc.tile_pool(name="psC", bufs=2, space="PSUM") as psC,
        ):
            wq_sb = wpool.tile([128, KT, HD], mmdt, tag="wq")
            wk_sb = wpool.tile([128, KT, HD], mmdt, tag="wk")
            wv_sb = wpool.tile([128, KT, HD], mmdt, tag="wv")
            wo_sb = wpool.tile([128, HD // 128, DIM], mmdt, tag="wo")
            eye_sb = wpool.tile([128, 128], mmdt, tag="eye")
            # x2T image goes first, k-chunked so the first KT matmul can
            # start after ~0.4MB instead of 2.5MB
            xk_sb = big.tile([128, KT, N], mmdt, tag="xkim")
            nc.sync.dma_start(wk_sb[:, 0, :], wk[:, 0, :])
            nc.sync.dma_start(xk_sb[:, 0, :], xk[:, 0, :])
            nc.sync.dma_start(wk_sb[:, 1:, :], wk[:, 1:, :])
            for k in range(1, KT):
                nc.sync.dma_start(xk_sb[:, k, :], xk[:, k, :])
            nc.sync.dma_start(wv_sb[:], wv[:])
            nc.sync.dma_start(eye_sb[:], eye[:])
            nc.sync.dma_start(wq_sb[:], wq[:])
            xq_sb = big.tile([128, KT, N], mmdt, tag="xqim")
            nc.sync.dma_start(xq_sb[:], xq[:])
            nc.sync.dma_start(wo_sb[:], wo[:])

            qt_sb = big.tile([128, HB, N], mmdt, tag="qt")
            kt_sb = big.tile([128, HB, N], mmdt, tag="kt")
            v_sb = big.tile([128, MT, H_c * 128], mmdt, tag="v")
            ct_sb = big.tile([128, HB, N], mmdt, tag="ct")

            if half_last:
                # the shared head's slot is only computed at local n0=0; its
                # n0=1 region of ctxT must read as zero in the out projection
                nc.gpsimd.memset(ct_sb[64:128, HB - 1, 512:N], 0.0)

            # ones columns of the augmented V blocks (parity-swapped per head)
            for h in range(H_c):
                c0 = h * 128 + (64 if h % 2 == 0 else 0)
                for m in range(MT):
                    nc.gpsimd.memset(v_sb[:, m, c0:c0 + 64], 1.0)

            # ---- Phase A: projections. One 1MB slab DMA per 512-column
            # block; KT and V share the x2T slabs. ----
            def v_copyout(ps_ap, m):
                pv = ps_ap.rearrange("p (hb two d) -> p hb two d", two=2, d=64)
                vv = v_sb[:, m, :].rearrange("p (hb x) -> p hb x", hb=HB)
                # even heads -> value cols 0:64 of their block; odd -> 192:256
                nc.vector.tensor_copy(vv[:, :, 0:64], pv[:, :, 0, :])
                nc.vector.tensor_copy(vv[:, :, 192:256], pv[:, :, 1, :])

            for m0 in range(NB):
                m_sl = slice(m0 * 512, (m0 + 1) * 512)
                slab = xk_sb[:, :, m_sl]
                kps = [psA.tile([128, 512], f32, tag="acc",
                                name=f"kps{m0}_{i}") for i in range(HB)]
                vps = [psS.tile([128, HD], f32, tag="sc",
                                name=f"vps{m0}_{i}") for i in range(2)]
                for k in range(KT):
                    for hb in range(HB):
                        nc.tensor.matmul(
                            kps[hb][:],
                            wk_sb[:, k, hb * 128:(hb + 1) * 128],
                            slab[:, k, :],
                            start=(k == 0), stop=(k == KT - 1),
                        )
                    for mi in range(2):
                        nc.tensor.matmul(
                            vps[mi][:],
                            slab[:, k, mi * 128:(mi + 1) * 128],
                            wv_sb[:, k, :],
                            start=(k == 0), stop=(k == KT - 1),
                        )
                for hb in range(HB):
                    nc.vector.tensor_copy(kt_sb[:, hb, m_sl], kps[hb][:])
                for mi in range(2):
                    v_copyout(vps[mi][:], m0 * 4 + mi)
                # second half of the V m-tiles from the same slab
                vpsb = [psS.tile([128, HD], f32, tag="sc",
                                 name=f"vpsb{m0}_{i}") for i in range(2)]
                for k in range(KT):
                    for mi in range(2):
                        nc.tensor.matmul(
                            vpsb[mi][:],
                            slab[:, k, (2 + mi) * 128:(3 + mi) * 128],
                            wv_sb[:, k, :],
                            start=(k == 0), stop=(k == KT - 1),
                        )
                for mi in range(2):
                    v_copyout(vpsb[mi][:], m0 * 4 + 2 + mi)

            for n0 in range(NB):
                n_sl = slice(n0 * 512, (n0 + 1) * 512)
                slab = xq_sb[:, :, n_sl]
                qps = [psA.tile([128, 512], f32, tag="acc",
                                name=f"psq{n0}_{i}") for i in range(HB)]
                for k in range(KT):
                    for hb in range(HB):
                        nc.tensor.matmul(
                            qps[hb][:],
                            wq_sb[:, k, hb * 128:(hb + 1) * 128],
                            slab[:, k, :],
                            start=(k == 0), stop=(k == KT - 1),
                        )
                for hb in range(HB):
                    nc.vector.tensor_copy(qt_sb[:, hb, n_sl], qps[hb][:])

            # ---- Phase B (attention) + C (out proj), interleaved per n-block ----
            for n0 in range(NB):
                n_sl = slice(n0 * 512, (n0 + 1) * 512)
                nheads = H_c - 1 if (half_last and n0 > 0) else H_c
                for h in range(nheads):
                    hb, hp = divmod(h, 2)
                    vb = hp * 64          # partition base of ctx values
                    sb_ = 64 - vb         # partition base of sumexp rows
                    ctx = psC.tile([128, 512], f32, tag="ctx")
                    spds = spdp.tile([128, MT, 512], mmdt, tag="spd")
                    nc.sync.dma_start(spds[:], spd[h, n0])
                    for m in range(MT):
                        sc = psS.tile([128, 512], f32, tag="sc")
                        nc.tensor.matmul(
                            sc[:],
                            kt_sb[vb:vb + 64, hb, m * 128:(m + 1) * 128],
                            qt_sb[vb:vb + 64, hb, n_sl],
                            start=True, stop=False,
                        )
                        # accumulate spd into the scores PSUM via identity MM
                        nc.tensor.matmul(
                            sc[:], eye_sb[:], spds[:, m, :], start=False, stop=True,
                        )
                        e = work.tile([128, 512], mmdt, tag="e")
                        nc.scalar.activation(e[:], sc[:], Exp)
                        nc.tensor.matmul(
                            ctx[:],
                            v_sb[:, m, h * 128:(h + 1) * 128],
                            e[:],
                            start=(m == 0),
                            stop=(m == MT - 1),
                        )
                    # normalize: ctxT = ctx_vals * keep_scale / sumexp (the
                    # ones block replicated sumexp across 64 rows at base sb_;
                    # copy to base 0 -- custom-DVE recip needs base 0 -- then
                    # stt against the PSUM value rows).
                    rr = work.tile([128, 512], f32, tag="rr")
                    if sb_ == 0:
                        nc.vector.reciprocal_approx_fast(
                            rr[0:64, :], ctx[0:64, :])
                    else:
                        ss = work.tile([128, 512], f32, tag="ss")
                        nc.vector.tensor_copy(ss[0:64, :], ctx[sb_:sb_ + 64, :])
                        nc.vector.reciprocal_approx_fast(rr[0:64, :], ss[0:64, :])
                    nc.vector.scalar_tensor_tensor(
                        out=ct_sb[vb:vb + 64, hb, n_sl],
                        in0=ctx[vb:vb + 64, :],
                        scalar=float(keep_scale),
                        in1=rr[0:64, :],
                        op0=mult,
                        op1=mult,
                    )

                # ---- Phase C for this n-block ----
                for nt in range(n0 * 4, (n0 + 1) * 4):
                    ot = work.tile([128, 2, 512], f32, tag="o")
                    for d0 in range(NB):
                        po = psA.tile([128, 512], f32, tag="acc",
                                      name=f"po{nt}_{d0}")
                        for kk in range(HD // 128):
                            nc.tensor.matmul(
                                po[:],
                                ct_sb[:, kk, nt * 128:(nt + 1) * 128],
                                wo_sb[:, kk, d0 * 512:(d0 + 1) * 512],
                                start=(kk == 0),
                                stop=(kk == HD // 128 - 1),
                            )
                        nc.vector.tensor_copy(ot[:, d0, :], po[:])
                    nc.sync.dma_start(
                        out[nt * 128:(nt + 1) * 128, :], ot[:])

    nc.finalize()
    return nc


def _get_nc(H_c: int, n_kept: int, mode: str, half_last: bool):
    key = (H_c, n_kept, mode, half_last)
    if key not in _cache:
        _cache[key] = _build(H_c, HEADS / n_kept, mode, half_last)
    return _cache[key]


def _prep_inputs(x1, x2, spd, head_keep, Wq, Wkv, Wo, mode="bf16"):
    """Slice/transpose/pad host-side into per-core input maps."""
    import ml_dtypes

    ndt = np.float32 if mode in ("f32", "f32r") else ml_dtypes.bfloat16
    kept = [int(i) for i in np.nonzero(head_keep)[0]]
    n_kept = len(kept)
    half_last = (n_kept % 2 == 1)
    if not half_last:
        H_c = n_kept // 2
        if H_c % 2:
            H_c += 1
        groups = [kept[:H_c], kept[H_c:]]
    else:
        # odd count: both cores of a pair share the last kept head, each
        # computing one n-half of it (local column order differs per core)
        K = (n_kept - 1) // 2
        shared = kept[-1]
        H_c = K + 1
        pad = []
        if H_c % 2:
            H_c += 1
            pad = [None]
        groups = [kept[:K] + pad + [shared], kept[K:2 * K] + pad + [shared]]

    Wk_full, Wv_full = Wkv[:, :INNER], Wkv[:, INNER:]

    in_maps = []
    for b in range(B):
        xqT = np.ascontiguousarray(
            x1[b].T.reshape(KT, 128, N).transpose(1, 0, 2)).astype(ndt)
        xkT = np.ascontiguousarray(
            x2[b].T.reshape(KT, 128, N).transpose(1, 0, 2)).astype(ndt)
        for g in range(2):
            heads = groups[g]
            swap = half_last and g == 1  # local n0=0 <-> global half 1
            xq_g = xqT
            if swap:
                xq_g = np.ascontiguousarray(
                    np.concatenate([xqT[:, :, 512:], xqT[:, :, :512]], axis=2))
            HD = H_c * DIM_HEAD
            wq_c = np.zeros((DIM, HD), np.float32)
            wk_c = np.zeros((DIM, HD), np.float32)
            wv_c = np.zeros((DIM, HD), np.float32)
            wo_c = np.zeros((HD, DIM), np.float32)
            spd_c = np.zeros((H_c, NB, 128, MT, 512), ndt)
            for i, h in enumerate(heads):
                if h is None:
                    continue
                sl = slice(i * DIM_HEAD, (i + 1) * DIM_HEAD)
                hs = slice(h * DIM_HEAD, (h + 1) * DIM_HEAD)
                wq_c[:, sl] = Wq[:, hs] * SCALE
                wk_c[:, sl] = Wk_full[:, hs]
                wv_c[:, sl] = Wv_full[:, hs]
                wo_c[sl, :] = Wo[hs, :]
                # spd[b,h].T -> [n-block, partition, m-tile, n] image,
                # n-blocks in the core's LOCAL column order
                im = (spd[b, h].T.reshape(MT, 128, NB, 512)
                      .transpose(2, 1, 0, 3)).astype(ndt)
                spd_c[i] = im[::-1] if swap else im
            in_maps.append({
                "xq": xq_g,
                "xk": xkT,
                "eye": np.eye(128, dtype=ndt),
                "wq": np.ascontiguousarray(
                    wq_c.reshape(KT, 128, HD).transpose(1, 0, 2)).astype(ndt),
                "wk": np.ascontiguousarray(
                    wk_c.reshape(KT, 128, HD).transpose(1, 0, 2)).astype(ndt),
                "wv": np.ascontiguousarray(
                    wv_c.reshape(KT, 128, HD).transpose(1, 0, 2)).astype(ndt),
                "wo": np.ascontiguousarray(
                    wo_c.reshape(HD // 128, 128, DIM).transpose(1, 0, 2)).astype(ndt),
                "spd": spd_c,
            })
    return in_maps, n_kept, H_c


def _run(nc, in_maps, trace=False, tmpdir=None):
    from concourse.bass_utils import run_bass_kernel_spmd

    return run_bass_kernel_spmd(
        nc, in_maps, core_ids=list(range(NCORES)), trace=trace, tmpdir=tmpdir
    )


def kernel(x1, x2, spd, head_keep, Wq, Wkv, Wo, bo, _trace=False, _tmpdir=None):
    x1 = np.asarray(x1, np.float32)
    x2 = np.asarray(x2, np.float32)
    spd = np.asarray(spd, np.float32)
    head_keep = np.asarray(head_keep)
    n_kept = int(head_keep.astype(np.int64).sum())
    if n_kept == 0:
        # reference: 16/0 = inf, 0*inf = nan everywhere
        return np.full((B, N, DIM), np.nan, np.float32)

    mode = os.environ.get("KERNEL_DTYPE", "bf16")
    in_maps, n_kept, H_c = _prep_inputs(
        x1, x2, spd, head_keep, Wq, Wkv, Wo, mode)
    half_last = (n_kept % 2 == 1)
    nc = _get_nc(H_c, n_kept, mode, half_last)
    res = _run(nc, in_maps, trace=_trace, tmpdir=_tmpdir)

    out = np.empty((B, N, DIM), np.float32)
    bo32 = np.asarray(bo, np.float32)
    for b in range(B):
        o0 = res.results[2 * b]["out"]
        o1 = res.results[2 * b + 1]["out"]
        if half_last:
            o1 = np.concatenate([o1[512:], o1[:512]], axis=0)
        out[b] = o0 + o1 + bo32
    kernel._last_results = res
    return out
